# revision 20
# baseline (speedup 1.0000x reference)
"""Color-preserving non-local block (dense softmax attention, N=9216, I=32)
distributed over 8 TRN2 NeuronCores.

Sharding: data-parallel over batch B=2 (4 cores per batch) x sequence-parallel
over the N=9216 query rows (2304 rows per core).  Each core receives the full
[C, N] image of its batch (rolled so its query slice starts at column 0 --
softmax over keys is permutation-invariant, so rolling the key axis is free),
computes the projections redundantly, and produces its [C, 2304] output slice.
No collectives are needed.

v3: every matmul uses a full K=128 contraction (K<128 streams at half clock on
this part).  theta/phi are projected with 4x-replicated weight matrices so the
QK matmul contracts over 4 redundant copies (St = 4x scores; the 1/4 folds
into the exp scale for free), x is sent twice-stacked on partitions for the
projections, and PV contracts over the 128-wide kv tile with a ones column
appended to g^T so the softmax denominator accumulates in PSUM row 32.
All matmuls are plain 128x128-mode (no tile_position -> no PE mode-switch
drains).  Per-chunk epilogues are deferred one chunk so the PE never waits on
the divide chain.

  main loop over q chunks (512) x kv tile pairs:
      QK:  2 plain matmuls  St[kv, q] = (phi4 tile)^T theta4     (233 ns each)
      exp: one ScalarE instr per pair: E = exp(St / (4 T)) -> bf16
      PV:  2 plain matmuls  Y[0:33, q] += gt_aug^T E   (PSUM accumulate)
"""

import sys

for _p in ("/opt/trn_rl_repo",):
    if _p not in sys.path:
        sys.path.insert(0, _p)

import numpy as np
import ml_dtypes

import concourse.bass as bass
import concourse.tile as tile
from concourse import bacc, mybir
from concourse.bass import ts, ds
from concourse.bass_utils import run_bass_kernel_spmd

F32 = mybir.dt.float32
BF16 = mybir.dt.bfloat16

B, C, H, W = 2, 64, 96, 96
N = H * W                    # 9216
I = 32                       # inter dim
NB = 16                      # gate bottleneck dim
NCORES = 8
CPB = NCORES // B            # cores per batch = 4
QPC = N // CPB               # 2304 query rows per core
KT = 128                     # kv tile
NKV = N // KT                # 72
GK = 3                       # kv tiles per St/exp group
NGR = NKV // GK              # 24 groups
QCH = 512                    # q chunk (PSUM free dim)
GTS = 34                     # gt free stride (33 used, kept 4B-aligned)
TEMP = 1.5
PR = 0.8


def _chunks():
    out = []
    q = 0
    while q < QPC:
        out.append((q, min(QCH, QPC - q)))
        q += QCH
    return out


def _emit(tc, nc, dr, out_d):
    mm = nc.tensor.matmul
    with (
        tc.tile_pool(name="consts", bufs=1) as consts,
        tc.tile_pool(name="work", bufs=2) as work,
        tc.tile_pool(name="epool", bufs=6) as epool,
    ):
        # ---- persistent SBUF tensors -------------------------------------
        xb_sb = consts.tile([C, N], F32)        # residual + gate path
        xbh2_sb = consts.tile([128, N], BF16)   # x stacked twice on partitions
        wbf_sb = consts.tile([128, 352], BF16)  # bf16 weight blob
        thw_sb = wbf_sb[:, 0:128]               # 0.5 * theta_w^T tiled (2, 4)
        phw_sb = wbf_sb[:, 128:256]             # 0.5 * phi_w^T tiled (2, 4)
        gw_sb = wbf_sb[:, 256:288]              # 0.5 * g_w^T tiled (2, 1)
        ww_sb = wbf_sb[:I, 288:352]             # W_w^T
        wf32_sb = consts.tile([C, 82], F32)     # f32 weight blob
        c1w_sb = wf32_sb[:, 0:NB]
        c1b_sb = wf32_sb[:NB, NB : NB + 1]
        c2w_sb = wf32_sb[:NB, 17:81]
        nc2b_sb = wf32_sb[:, 81:82]

        theta4_sb = consts.tile([128, QPC], BF16)   # theta replicated x4
        phi4_sb = consts.tile([128, N], BF16)       # phi replicated x4
        gt_sb = consts.tile([128, NKV, GTS], BF16)  # [kv, tile, i | ones | pad]
        gate_sb = consts.tile([C, 1], F32)
        pool_sb = consts.tile([C, 1], F32)
        h_sb = consts.tile([NB, 1], F32)
        eg_sb = consts.tile([C, 1], F32)

        nc.sync.dma_start(out=wbf_sb, in_=dr["wbf"])
        nc.sync.dma_start(out=xbh2_sb[:, :QPC], in_=dr["xbh2"][:, :QPC])
        nc.sync.dma_start(out=xbh2_sb[:, QPC:], in_=dr["xbh2"][:, QPC:])
        nc.scalar.dma_start(out=xb_sb, in_=dr["xb"])
        nc.scalar.dma_start(out=wf32_sb, in_=dr["wf32"])

        ones72 = consts.tile([128, NKV], F32)
        nc.vector.memset(ones72, 1.0)
        nc.vector.tensor_copy(out=gt_sb[:, :, I], in_=ones72)

        # ---- prologue projections (all K=128) ----------------------------
        with tc.tile_pool(name="ppsum", bufs=4, space="PSUM") as pp:
            for pi, (qs, qn) in enumerate(_chunks()):
                pt = pp.tile([128, QCH], F32, tag="pp")
                mm(out=pt[:, :qn], lhsT=thw_sb, rhs=xbh2_sb[:, ds(qs, qn)],
                   start=True, stop=True)
                if pi % 2 == 0:
                    nc.scalar.copy(out=theta4_sb[:, ds(qs, qn)], in_=pt[:, :qn])
                else:
                    nc.vector.tensor_copy(out=theta4_sb[:, ds(qs, qn)],
                                          in_=pt[:, :qn])
            for c in range(N // QCH):
                pt = pp.tile([128, QCH], F32, tag="pp")
                mm(out=pt, lhsT=phw_sb, rhs=xbh2_sb[:, ts(c, QCH)],
                   start=True, stop=True)
                if c % 2 == 0:
                    nc.scalar.copy(out=phi4_sb[:, ts(c, QCH)], in_=pt)
                else:
                    nc.vector.tensor_copy(out=phi4_sb[:, ts(c, QCH)], in_=pt)
            done = 0
            while done < NKV:
                nt = min(16, NKV - done)
                pt = pp.tile([128, QCH], F32, tag="pp")
                for k in range(nt):
                    t = done + k
                    mm(out=pt[:, ts(k, I)], lhsT=xbh2_sb[:, ts(t, KT)],
                       rhs=gw_sb, start=True, stop=True)
                nc.vector.tensor_copy(
                    out=gt_sb[:, done : done + nt, :I],
                    in_=pt[:, : nt * I].rearrange("p (k i) -> p k i", i=I),
                )
                done += nt

        # ---- main loop ---------------------------------------------------
        with (
            tc.tile_pool(name="pst", bufs=2, space="PSUM") as pst,
            tc.tile_pool(name="py", bufs=1, space="PSUM") as py,
            tc.tile_pool(name="pmisc", bufs=1, space="PSUM") as pmisc,
        ):
            def emit_gate():
                # channel gate; emitted after chunk 0's pairs so its matmuls
                # (which wait on the DVE mean-reduce) never block the PE queue
                # ahead of the main stream
                nc.vector.reduce_sum(out=pool_sb, in_=xb_sb,
                                     axis=mybir.AxisListType.X)
                h_ps = pmisc.tile([128, QCH], F32, tag="m")
                mm(out=h_ps[:NB, 0:1], lhsT=c1w_sb, rhs=pool_sb,
                   start=True, stop=True)
                nc.scalar.activation(out=h_sb, in_=h_ps[:NB, 0:1],
                                     func=mybir.ActivationFunctionType.Relu,
                                     bias=c1b_sb, scale=1.0 / float(N))
                z_ps = pmisc.tile([128, QCH], F32, tag="m")
                mm(out=z_ps[:C, 0:1], lhsT=c2w_sb, rhs=h_sb,
                   start=True, stop=True)
                nc.scalar.activation(out=eg_sb, in_=z_ps[:C, 0:1],
                                     func=mybir.ActivationFunctionType.Exp,
                                     bias=nc2b_sb, scale=-1.0)
                nc.vector.tensor_scalar_add(gate_sb, eg_sb, 1.0)
                nc.vector.reciprocal(out=gate_sb, in_=gate_sb)
                nc.vector.tensor_scalar_mul(gate_sb, gate_sb, PR)

            pending = None
            for ci, (qs, qn) in enumerate(_chunks()):
                y_ps = py.tile([I + 1, QCH], F32, tag="y")
                for g in range(NGR):
                    # the previous chunk's PE tail goes here, a few groups in,
                    # so its divide chain has finished on DVE by now
                    if g == 6 and pending is not None:
                        pending()
                        pending = None
                    st = pst.tile([128, GK, QCH], F32, tag="st")
                    for j in range(GK):
                        t = GK * g + j
                        mm(out=st[:, j, :qn],
                           lhsT=phi4_sb[:, ts(t, KT)],
                           rhs=theta4_sb[:, ds(qs, qn)],
                           start=True, stop=True)
                    e_t = epool.tile([128, GK, QCH], BF16, tag="e")
                    nc.scalar.activation(out=e_t[:, :, :qn], in_=st[:, :, :qn],
                                         func=mybir.ActivationFunctionType.Exp,
                                         scale=1.0 / (4.0 * TEMP))
                    for j in range(GK):
                        t = GK * g + j
                        mm(out=y_ps[:, :qn],
                           lhsT=gt_sb[:, t, : I + 1],
                           rhs=e_t[:, j, :qn],
                           start=(t == 0), stop=(t == NKV - 1))
                if ci == 0:
                    emit_gate()
                # epilogue: copy Y out (frees the bank), W-project the
                # UNNORMALIZED Y (so the PE tail never waits on the divide),
                # and fold 1/denominator into the final DVE pass
                last = qs + qn >= QPC

                def _epi_head(q0, q1, y_ps=y_ps):
                    n = q1 - q0
                    ysum = work.tile([I, QCH], BF16, tag="ysum")
                    nc.vector.tensor_copy(out=ysum[:, :n],
                                          in_=y_ps[:I, q0:q1])
                    d_sb = work.tile([1, QCH], F32, tag="d")
                    nc.vector.tensor_copy(out=d_sb[:, :n],
                                          in_=y_ps[I : I + 1, q0:q1])
                    recip = work.tile([1, QCH], F32, tag="recip")
                    nc.vector.reciprocal(out=recip[:, :n], in_=d_sb[:, :n])
                    bc = work.tile([C, QCH], F32, tag="bc")
                    nc.gpsimd.partition_broadcast(bc[:, :n], recip[:, :n])
                    return ysum, bc

                def _epi_tail(q0, q1, ysum, bc, qs=qs):
                    n = q1 - q0
                    o_ps = pmisc.tile([128, QCH], F32, tag="m")
                    mm(out=o_ps[:C, :n], lhsT=ww_sb, rhs=ysum[:, :n],
                       start=True, stop=True)
                    t1 = work.tile([C, QCH], F32, tag="t1")
                    nc.vector.tensor_mul(t1[:, :n], o_ps[:C, :n], bc[:, :n])
                    out_sb = work.tile([C, QCH], F32, tag="out")
                    nc.vector.scalar_tensor_tensor(
                        out=out_sb[:, :n], in0=t1[:, :n], scalar=gate_sb,
                        in1=xb_sb[:, ds(qs + q0, n)],
                        op0=mybir.AluOpType.mult, op1=mybir.AluOpType.add)
                    nc.sync.dma_start(out=out_d[:, ds(qs + q0, n)],
                                      in_=out_sb[:, :n])

                if not last:
                    ysum, bc = _epi_head(0, qn)

                    def _tail(qs=qs, qn=qn, ysum=ysum, bc=bc):
                        _epi_tail(0, qn, ysum, bc, qs=qs)

                    pending = _tail
                else:
                    # final chunk: two pipelined half-epilogues to shorten
                    # the serial tail
                    h = qn // 2
                    ya, ba = _epi_head(0, h)
                    _epi_tail(0, h, ya, ba)
                    yb, bb = _epi_head(h, qn)
                    _epi_tail(h, qn, yb, bb)
            if pending is not None:
                pending()


def build():
    nc = bacc.Bacc("TRN2", target_bir_lowering=False, debug=False)
    names = {
        "xb": ([C, N], F32), "xbh2": ([128, N], BF16),
        "wbf": ([128, 352], BF16), "wf32": ([C, 82], F32),
    }
    dr = {k: nc.dram_tensor(k, shp, dt, kind="ExternalInput").ap()
          for k, (shp, dt) in names.items()}
    out_d = nc.dram_tensor("out", [C, QPC], F32, kind="ExternalOutput").ap()
    with tile.TileContext(nc) as tc:
        _emit(tc, nc, dr, out_d)
    nc.compile()
    return nc


_NC = None


def _get_nc():
    global _NC
    if _NC is None:
        _NC = build()
    return _NC


def make_in_maps(inputs):
    bf = ml_dtypes.bfloat16
    xf = np.ascontiguousarray(np.asarray(inputs["x"], np.float32).reshape(B, C, N))
    thwT = np.asarray(inputs["theta_w"], np.float32).T        # [C, I]
    phwT = np.asarray(inputs["phi_w"], np.float32).T
    gwT = np.asarray(inputs["g_w"], np.float32).T
    wbf = np.zeros((128, 352), np.float32)
    wbf[:, 0:128] = np.tile(thwT, (2, 4)) * 0.5
    wbf[:, 128:256] = np.tile(phwT, (2, 4)) * 0.5
    wbf[:, 256:288] = np.tile(gwT, (2, 1)) * 0.5
    wbf[:I, 288:352] = np.asarray(inputs["W_w"], np.float32).T
    wf32 = np.zeros((C, 82), np.float32)
    wf32[:, 0:NB] = np.asarray(inputs["cg1_w"], np.float32).T
    wf32[:NB, NB] = np.asarray(inputs["cg1_b"], np.float32)
    wf32[:NB, 17:81] = np.asarray(inputs["cg2_w"], np.float32).T
    wf32[:, 81] = -np.asarray(inputs["cg2_b"], np.float32)
    shared = {"wbf": wbf.astype(bf), "wf32": wf32}
    in_maps = []
    for core in range(NCORES):
        b, q0 = core // CPB, (core % CPB) * QPC
        m = dict(shared)
        xr = np.ascontiguousarray(np.roll(xf[b], -q0, axis=1))
        m["xb"] = xr
        m["xbh2"] = np.ascontiguousarray(np.tile(xr, (2, 1))).astype(bf)
        in_maps.append(m)
    return in_maps


def gather(results):
    y = np.empty((B, C, N), np.float32)
    for core in range(NCORES):
        b, q0 = core // CPB, (core % CPB) * QPC
        y[b][:, q0 : q0 + QPC] = results[core]["out"]
    return y.reshape(B, C, H, W)


def run(inputs, trace=False, **kw):
    res = run_bass_kernel_spmd(_get_nc(), make_in_maps(inputs),
                               core_ids=list(range(NCORES)), trace=trace, **kw)
    return gather(res.results), res


def kernel(**inputs):
    out, _ = run(inputs)
    return out


# revision 21
# speedup vs baseline: 1.0168x; 1.0168x over previous
"""Color-preserving non-local block (dense softmax attention, N=9216, I=32)
distributed over 8 TRN2 NeuronCores.

Sharding: data-parallel over batch B=2 (4 cores per batch) x sequence-parallel
over the N=9216 query rows (2304 rows per core).  Each core receives the full
[C, N] image of its batch (rolled so its query slice starts at column 0 --
softmax over keys is permutation-invariant, so rolling the key axis is free),
computes the projections redundantly, and produces its [C, 2304] output slice.
No collectives are needed.

v3: every matmul uses a full K=128 contraction (K<128 streams at half clock on
this part).  theta/phi are projected with 4x-replicated weight matrices so the
QK matmul contracts over 4 redundant copies (St = 4x scores; the 1/4 folds
into the exp scale for free), x is sent twice-stacked on partitions for the
projections, and PV contracts over the 128-wide kv tile with a ones column
appended to g^T so the softmax denominator accumulates in PSUM row 32.
All matmuls are plain 128x128-mode (no tile_position -> no PE mode-switch
drains).  Per-chunk epilogues are deferred one chunk so the PE never waits on
the divide chain.

  main loop over q chunks (512) x kv tile pairs:
      QK:  2 plain matmuls  St[kv, q] = (phi4 tile)^T theta4     (233 ns each)
      exp: one ScalarE instr per pair: E = exp(St / (4 T)) -> bf16
      PV:  2 plain matmuls  Y[0:33, q] += gt_aug^T E   (PSUM accumulate)
"""

import sys

for _p in ("/opt/trn_rl_repo",):
    if _p not in sys.path:
        sys.path.insert(0, _p)

import numpy as np
import ml_dtypes

import concourse.bass as bass
import concourse.tile as tile
from concourse import bacc, mybir
from concourse.bass import ts, ds
from concourse.bass_utils import run_bass_kernel_spmd

F32 = mybir.dt.float32
BF16 = mybir.dt.bfloat16

B, C, H, W = 2, 64, 96, 96
N = H * W                    # 9216
I = 32                       # inter dim
NB = 16                      # gate bottleneck dim
NCORES = 8
CPB = NCORES // B            # cores per batch = 4
QPC = N // CPB               # 2304 query rows per core
KT = 128                     # kv tile
NKV = N // KT                # 72
GK = 3                       # kv tiles per St/exp group
NGR = NKV // GK              # 24 groups
QCH = 512                    # q chunk (PSUM free dim)
GTS = 34                     # gt free stride (33 used, kept 4B-aligned)
TEMP = 1.5
PR = 0.8


def _chunks():
    out = []
    q = 0
    while q < QPC:
        out.append((q, min(QCH, QPC - q)))
        q += QCH
    return out


def _emit(tc, nc, dr, out_d):
    mm = nc.tensor.matmul
    with (
        tc.tile_pool(name="consts", bufs=1) as consts,
        tc.tile_pool(name="work", bufs=2) as work,
        tc.tile_pool(name="epool", bufs=6) as epool,
    ):
        # ---- persistent SBUF tensors -------------------------------------
        xb_sb = consts.tile([C, N], F32)        # residual + gate path
        xbh2_sb = consts.tile([128, N], BF16)   # x stacked twice on partitions
        wbf_sb = consts.tile([128, 352], BF16)  # bf16 weight blob
        thw_sb = wbf_sb[:, 0:128]               # 0.5 * theta_w^T tiled (2, 4)
        phw_sb = wbf_sb[:, 128:256]             # 0.5 * phi_w^T tiled (2, 4)
        gw_sb = wbf_sb[:, 256:288]              # 0.5 * g_w^T tiled (2, 1)
        ww_sb = wbf_sb[:I, 288:352]             # W_w^T
        wf32_sb = consts.tile([C, 82], F32)     # f32 weight blob
        c1w_sb = wf32_sb[:, 0:NB]
        c1b_sb = wf32_sb[:NB, NB : NB + 1]
        c2w_sb = wf32_sb[:NB, 17:81]
        nc2b_sb = wf32_sb[:, 81:82]

        theta4_sb = consts.tile([128, QPC], BF16)   # theta replicated x4
        phi4_sb = consts.tile([128, N], BF16)       # phi replicated x4
        gt_sb = consts.tile([128, NKV, GTS], BF16)  # [kv, tile, i | ones | pad]
        gate_sb = consts.tile([C, 1], F32)
        pool_sb = consts.tile([C, 1], F32)
        h_sb = consts.tile([NB, 1], F32)
        eg_sb = consts.tile([C, 1], F32)

        nc.sync.dma_start(out=wbf_sb, in_=dr["wbf"])
        nc.sync.dma_start(out=xbh2_sb[:, :QPC], in_=dr["xbh2"][:, :QPC])
        nc.sync.dma_start(out=xbh2_sb[:, QPC:], in_=dr["xbh2"][:, QPC:])
        nc.scalar.dma_start(out=xb_sb, in_=dr["xb"])
        nc.scalar.dma_start(out=wf32_sb, in_=dr["wf32"])

        ones72 = consts.tile([128, NKV], F32)
        nc.vector.memset(ones72, 1.0)
        nc.vector.tensor_copy(out=gt_sb[:, :, I], in_=ones72)

        # ---- prologue projections (all K=128) ----------------------------
        with tc.tile_pool(name="ppsum", bufs=4, space="PSUM") as pp:
            for qs, qn in _chunks():
                pt = pp.tile([128, QCH], F32, tag="pp")
                mm(out=pt[:, :qn], lhsT=thw_sb, rhs=xbh2_sb[:, ds(qs, qn)],
                   start=True, stop=True)
                nc.scalar.copy(out=theta4_sb[:, ds(qs, qn)], in_=pt[:, :qn])
            for c in range(N // QCH):
                pt = pp.tile([128, QCH], F32, tag="pp")
                mm(out=pt, lhsT=phw_sb, rhs=xbh2_sb[:, ts(c, QCH)],
                   start=True, stop=True)
                nc.scalar.copy(out=phi4_sb[:, ts(c, QCH)], in_=pt)
            done = 0
            while done < NKV:
                nt = min(16, NKV - done)
                pt = pp.tile([128, QCH], F32, tag="pp")
                for k in range(nt):
                    t = done + k
                    mm(out=pt[:, ts(k, I)], lhsT=xbh2_sb[:, ts(t, KT)],
                       rhs=gw_sb, start=True, stop=True)
                nc.vector.tensor_copy(
                    out=gt_sb[:, done : done + nt, :I],
                    in_=pt[:, : nt * I].rearrange("p (k i) -> p k i", i=I),
                )
                done += nt

        # ---- main loop ---------------------------------------------------
        with (
            tc.tile_pool(name="pst", bufs=2, space="PSUM") as pst,
            tc.tile_pool(name="py", bufs=1, space="PSUM") as py,
            tc.tile_pool(name="pmisc", bufs=1, space="PSUM") as pmisc,
        ):
            def emit_gate():
                # channel gate; emitted after chunk 0's pairs so its matmuls
                # (which wait on the DVE mean-reduce) never block the PE queue
                # ahead of the main stream
                nc.vector.reduce_sum(out=pool_sb, in_=xb_sb,
                                     axis=mybir.AxisListType.X)
                h_ps = pmisc.tile([128, QCH], F32, tag="m")
                mm(out=h_ps[:NB, 0:1], lhsT=c1w_sb, rhs=pool_sb,
                   start=True, stop=True)
                nc.scalar.activation(out=h_sb, in_=h_ps[:NB, 0:1],
                                     func=mybir.ActivationFunctionType.Relu,
                                     bias=c1b_sb, scale=1.0 / float(N))
                z_ps = pmisc.tile([128, QCH], F32, tag="m")
                mm(out=z_ps[:C, 0:1], lhsT=c2w_sb, rhs=h_sb,
                   start=True, stop=True)
                nc.scalar.activation(out=eg_sb, in_=z_ps[:C, 0:1],
                                     func=mybir.ActivationFunctionType.Exp,
                                     bias=nc2b_sb, scale=-1.0)
                nc.vector.tensor_scalar_add(gate_sb, eg_sb, 1.0)
                nc.vector.reciprocal(out=gate_sb, in_=gate_sb)
                nc.vector.tensor_scalar_mul(gate_sb, gate_sb, PR)

            pending = None
            for ci, (qs, qn) in enumerate(_chunks()):
                y_ps = py.tile([I + 1, QCH], F32, tag="y")
                for g in range(NGR):
                    # the previous chunk's PE tail goes here, a few groups in,
                    # so its divide chain has finished on DVE by now
                    if g == 6 and pending is not None:
                        pending()
                        pending = None
                    st = pst.tile([128, GK, QCH], F32, tag="st")
                    for j in range(GK):
                        t = GK * g + j
                        mm(out=st[:, j, :qn],
                           lhsT=phi4_sb[:, ts(t, KT)],
                           rhs=theta4_sb[:, ds(qs, qn)],
                           start=True, stop=True)
                    e_t = epool.tile([128, GK, QCH], BF16, tag="e")
                    nc.scalar.activation(out=e_t[:, :, :qn], in_=st[:, :, :qn],
                                         func=mybir.ActivationFunctionType.Exp,
                                         scale=1.0 / (4.0 * TEMP))
                    for j in range(GK):
                        t = GK * g + j
                        mm(out=y_ps[:, :qn],
                           lhsT=gt_sb[:, t, : I + 1],
                           rhs=e_t[:, j, :qn],
                           start=(t == 0), stop=(t == NKV - 1))
                if ci == 0:
                    emit_gate()
                # epilogue: copy Y out (frees the bank), W-project the
                # UNNORMALIZED Y (so the PE tail never waits on the divide),
                # and fold 1/denominator into the final DVE pass
                last = qs + qn >= QPC

                def _epi_head(q0, q1, y_ps=y_ps):
                    n = q1 - q0
                    ysum = work.tile([I, QCH], BF16, tag="ysum")
                    nc.vector.tensor_copy(out=ysum[:, :n],
                                          in_=y_ps[:I, q0:q1])
                    d_sb = work.tile([1, QCH], F32, tag="d")
                    nc.vector.tensor_copy(out=d_sb[:, :n],
                                          in_=y_ps[I : I + 1, q0:q1])
                    recip = work.tile([1, QCH], F32, tag="recip")
                    nc.vector.reciprocal(out=recip[:, :n], in_=d_sb[:, :n])
                    bc = work.tile([C, QCH], F32, tag="bc")
                    nc.gpsimd.partition_broadcast(bc[:, :n], recip[:, :n])
                    return ysum, bc

                def _epi_tail(q0, q1, ysum, bc, qs=qs):
                    n = q1 - q0
                    o_ps = pmisc.tile([128, QCH], F32, tag="m")
                    mm(out=o_ps[:C, :n], lhsT=ww_sb, rhs=ysum[:, :n],
                       start=True, stop=True)
                    t1 = work.tile([C, QCH], F32, tag="t1")
                    nc.vector.tensor_mul(t1[:, :n], o_ps[:C, :n], bc[:, :n])
                    out_sb = work.tile([C, QCH], F32, tag="out")
                    nc.vector.scalar_tensor_tensor(
                        out=out_sb[:, :n], in0=t1[:, :n], scalar=gate_sb,
                        in1=xb_sb[:, ds(qs + q0, n)],
                        op0=mybir.AluOpType.mult, op1=mybir.AluOpType.add)
                    nc.sync.dma_start(out=out_d[:, ds(qs + q0, n)],
                                      in_=out_sb[:, :n])

                if not last:
                    ysum, bc = _epi_head(0, qn)

                    def _tail(qs=qs, qn=qn, ysum=ysum, bc=bc):
                        _epi_tail(0, qn, ysum, bc, qs=qs)

                    pending = _tail
                else:
                    # final chunk: two pipelined half-epilogues to shorten
                    # the serial tail
                    h = qn // 2
                    ya, ba = _epi_head(0, h)
                    _epi_tail(0, h, ya, ba)
                    yb, bb = _epi_head(h, qn)
                    _epi_tail(h, qn, yb, bb)
            if pending is not None:
                pending()


def build():
    nc = bacc.Bacc("TRN2", target_bir_lowering=False, debug=False)
    names = {
        "xb": ([C, N], F32), "xbh2": ([128, N], BF16),
        "wbf": ([128, 352], BF16), "wf32": ([C, 82], F32),
    }
    dr = {k: nc.dram_tensor(k, shp, dt, kind="ExternalInput").ap()
          for k, (shp, dt) in names.items()}
    out_d = nc.dram_tensor("out", [C, QPC], F32, kind="ExternalOutput").ap()
    with tile.TileContext(nc) as tc:
        _emit(tc, nc, dr, out_d)
    nc.compile()
    return nc


_NC = None


def _get_nc():
    global _NC
    if _NC is None:
        _NC = build()
    return _NC


def make_in_maps(inputs):
    bf = ml_dtypes.bfloat16
    xf = np.ascontiguousarray(np.asarray(inputs["x"], np.float32).reshape(B, C, N))
    thwT = np.asarray(inputs["theta_w"], np.float32).T        # [C, I]
    phwT = np.asarray(inputs["phi_w"], np.float32).T
    gwT = np.asarray(inputs["g_w"], np.float32).T
    wbf = np.zeros((128, 352), np.float32)
    wbf[:, 0:128] = np.tile(thwT, (2, 4)) * 0.5
    wbf[:, 128:256] = np.tile(phwT, (2, 4)) * 0.5
    wbf[:, 256:288] = np.tile(gwT, (2, 1)) * 0.5
    wbf[:I, 288:352] = np.asarray(inputs["W_w"], np.float32).T
    wf32 = np.zeros((C, 82), np.float32)
    wf32[:, 0:NB] = np.asarray(inputs["cg1_w"], np.float32).T
    wf32[:NB, NB] = np.asarray(inputs["cg1_b"], np.float32)
    wf32[:NB, 17:81] = np.asarray(inputs["cg2_w"], np.float32).T
    wf32[:, 81] = -np.asarray(inputs["cg2_b"], np.float32)
    shared = {"wbf": wbf.astype(bf), "wf32": wf32}
    in_maps = []
    for core in range(NCORES):
        b, q0 = core // CPB, (core % CPB) * QPC
        m = dict(shared)
        xr = np.ascontiguousarray(np.roll(xf[b], -q0, axis=1))
        m["xb"] = xr
        m["xbh2"] = np.ascontiguousarray(np.tile(xr, (2, 1))).astype(bf)
        in_maps.append(m)
    return in_maps


def gather(results):
    y = np.empty((B, C, N), np.float32)
    for core in range(NCORES):
        b, q0 = core // CPB, (core % CPB) * QPC
        y[b][:, q0 : q0 + QPC] = results[core]["out"]
    return y.reshape(B, C, H, W)


def run(inputs, trace=False, **kw):
    res = run_bass_kernel_spmd(_get_nc(), make_in_maps(inputs),
                               core_ids=list(range(NCORES)), trace=trace, **kw)
    return gather(res.results), res


def kernel(**inputs):
    out, _ = run(inputs)
    return out


# revision 22
# speedup vs baseline: 1.0222x; 1.0053x over previous
"""Color-preserving non-local block (dense softmax attention, N=9216, I=32)
distributed over 8 TRN2 NeuronCores.

Sharding: data-parallel over batch B=2 (4 cores per batch) x sequence-parallel
over the N=9216 query rows (2304 rows per core).  Each core receives the full
[C, N] image of its batch (rolled so its query slice starts at column 0 --
softmax over keys is permutation-invariant, so rolling the key axis is free),
computes the projections redundantly, and produces its [C, 2304] output slice.
No collectives are needed.

v3: every matmul uses a full K=128 contraction (K<128 streams at half clock on
this part).  theta/phi are projected with 4x-replicated weight matrices so the
QK matmul contracts over 4 redundant copies (St = 4x scores; the 1/4 folds
into the exp scale for free), x is sent twice-stacked on partitions for the
projections, and PV contracts over the 128-wide kv tile with a ones column
appended to g^T so the softmax denominator accumulates in PSUM row 32.
All matmuls are plain 128x128-mode (no tile_position -> no PE mode-switch
drains).  Per-chunk epilogues are deferred one chunk so the PE never waits on
the divide chain.

  main loop over q chunks (512) x kv tile pairs:
      QK:  2 plain matmuls  St[kv, q] = (phi4 tile)^T theta4     (233 ns each)
      exp: one ScalarE instr per pair: E = exp(St / (4 T)) -> bf16
      PV:  2 plain matmuls  Y[0:33, q] += gt_aug^T E   (PSUM accumulate)
"""

import sys

for _p in ("/opt/trn_rl_repo",):
    if _p not in sys.path:
        sys.path.insert(0, _p)

import numpy as np
import ml_dtypes

import concourse.bass as bass
import concourse.tile as tile
from concourse import bacc, mybir
from concourse.bass import ts, ds
from concourse.bass_utils import run_bass_kernel_spmd

F32 = mybir.dt.float32
BF16 = mybir.dt.bfloat16

B, C, H, W = 2, 64, 96, 96
N = H * W                    # 9216
I = 32                       # inter dim
NB = 16                      # gate bottleneck dim
NCORES = 8
CPB = NCORES // B            # cores per batch = 4
QPC = N // CPB               # 2304 query rows per core
KT = 128                     # kv tile
NKV = N // KT                # 72
GK = 3                       # kv tiles per St/exp group
NGR = NKV // GK              # 24 groups
QCH = 512                    # q chunk (PSUM free dim)
GTS = 34                     # gt free stride (33 used, kept 4B-aligned)
TEMP = 1.5
PR = 0.8


def _chunks():
    out = []
    q = 0
    while q < QPC:
        out.append((q, min(QCH, QPC - q)))
        q += QCH
    return out


def _emit(tc, nc, dr, out_d):
    mm = nc.tensor.matmul
    with (
        tc.tile_pool(name="consts", bufs=1) as consts,
        tc.tile_pool(name="work", bufs=2) as work,
        tc.tile_pool(name="epool", bufs=6) as epool,
    ):
        # ---- persistent SBUF tensors -------------------------------------
        xb_sb = consts.tile([C, N], F32)        # residual + gate path
        xbh2_sb = consts.tile([128, N], BF16)   # x stacked twice on partitions
        wbf_sb = consts.tile([128, 352], BF16)  # bf16 weight blob
        thw_sb = wbf_sb[:, 0:128]               # 0.5 * theta_w^T tiled (2, 4)
        phw_sb = wbf_sb[:, 128:256]             # 0.5 * phi_w^T tiled (2, 4)
        gw_sb = wbf_sb[:, 256:288]              # 0.5 * g_w^T tiled (2, 1)
        ww_sb = wbf_sb[:I, 288:352]             # W_w^T
        wf32_sb = consts.tile([C, 82], F32)     # f32 weight blob
        c1w_sb = wf32_sb[:, 0:NB]
        c1b_sb = wf32_sb[:NB, NB : NB + 1]
        c2w_sb = wf32_sb[:NB, 17:81]
        nc2b_sb = wf32_sb[:, 81:82]

        theta4_sb = consts.tile([128, QPC], BF16)   # theta replicated x4
        phi4_sb = consts.tile([128, N], BF16)       # phi replicated x4
        gt_sb = consts.tile([128, NKV, GTS], BF16)  # [kv, tile, i | ones | pad]
        gate_sb = consts.tile([C, 1], F32)
        pool_sb = consts.tile([C, 1], F32)
        h_sb = consts.tile([NB, 1], F32)
        eg_sb = consts.tile([C, 1], F32)

        nc.sync.dma_start(out=wbf_sb, in_=dr["wbf"])
        nc.sync.dma_start(out=xbh2_sb[:, :QPC], in_=dr["xbh2"][:, :QPC])
        nc.sync.dma_start(out=xbh2_sb[:, QPC:], in_=dr["xbh2"][:, QPC:])
        nc.scalar.dma_start(out=xb_sb, in_=dr["xb"])
        nc.scalar.dma_start(out=wf32_sb, in_=dr["wf32"])

        ones72 = consts.tile([128, NKV], F32)
        nc.vector.memset(ones72, 1.0)
        nc.vector.tensor_copy(out=gt_sb[:, :, I], in_=ones72)

        # ---- prologue projections (all K=128) ----------------------------
        with tc.tile_pool(name="ppsum", bufs=4, space="PSUM") as pp:
            for qs, qn in _chunks():
                pt = pp.tile([128, QCH], F32, tag="pp")
                mm(out=pt[:, :qn], lhsT=thw_sb, rhs=xbh2_sb[:, ds(qs, qn)],
                   start=True, stop=True)
                nc.scalar.copy(out=theta4_sb[:, ds(qs, qn)], in_=pt[:, :qn])
            for c in range(N // QCH):
                pt = pp.tile([128, QCH], F32, tag="pp")
                mm(out=pt, lhsT=phw_sb, rhs=xbh2_sb[:, ts(c, QCH)],
                   start=True, stop=True)
                nc.scalar.copy(out=phi4_sb[:, ts(c, QCH)], in_=pt)
            done = 0
            while done < NKV:
                nt = min(16, NKV - done)
                pt = pp.tile([128, QCH], F32, tag="pp")
                for k in range(nt):
                    t = done + k
                    mm(out=pt[:, ts(k, I)], lhsT=xbh2_sb[:, ts(t, KT)],
                       rhs=gw_sb, start=True, stop=True)
                nc.vector.tensor_copy(
                    out=gt_sb[:, done : done + nt, :I],
                    in_=pt[:, : nt * I].rearrange("p (k i) -> p k i", i=I),
                )
                done += nt

        # ---- main loop ---------------------------------------------------
        with (
            tc.tile_pool(name="pst", bufs=2, space="PSUM") as pst,
            tc.tile_pool(name="py", bufs=1, space="PSUM") as py,
            tc.tile_pool(name="pmisc", bufs=1, space="PSUM") as pmisc,
        ):
            def emit_gate():
                # channel gate; emitted after chunk 0's pairs so its matmuls
                # (which wait on the DVE mean-reduce) never block the PE queue
                # ahead of the main stream
                nc.vector.reduce_sum(out=pool_sb, in_=xb_sb,
                                     axis=mybir.AxisListType.X)
                h_ps = pmisc.tile([128, QCH], F32, tag="m")
                mm(out=h_ps[:NB, 0:1], lhsT=c1w_sb, rhs=pool_sb,
                   start=True, stop=True)
                nc.scalar.activation(out=h_sb, in_=h_ps[:NB, 0:1],
                                     func=mybir.ActivationFunctionType.Relu,
                                     bias=c1b_sb, scale=1.0 / float(N))
                z_ps = pmisc.tile([128, QCH], F32, tag="m")
                mm(out=z_ps[:C, 0:1], lhsT=c2w_sb, rhs=h_sb,
                   start=True, stop=True)
                nc.scalar.activation(out=eg_sb, in_=z_ps[:C, 0:1],
                                     func=mybir.ActivationFunctionType.Exp,
                                     bias=nc2b_sb, scale=-1.0)
                nc.vector.tensor_scalar_add(gate_sb, eg_sb, 1.0)
                nc.vector.reciprocal(out=gate_sb, in_=gate_sb)
                nc.vector.tensor_scalar_mul(gate_sb, gate_sb, PR)

            pending = None
            for ci, (qs, qn) in enumerate(_chunks()):
                y_ps = py.tile([I + 1, QCH], F32, tag="y")
                for g in range(NGR):
                    # the previous chunk's PE tail goes here, a few groups in,
                    # so its divide chain has finished on DVE by now
                    if g == 6 and pending is not None:
                        pending()
                        pending = None
                    st = pst.tile([128, GK, QCH], F32, tag="st")
                    for j in range(GK):
                        t = GK * g + j
                        mm(out=st[:, j, :qn],
                           lhsT=phi4_sb[:, ts(t, KT)],
                           rhs=theta4_sb[:, ds(qs, qn)],
                           start=True, stop=True)
                    e_t = epool.tile([128, GK, QCH], BF16, tag="e")
                    nc.scalar.activation(out=e_t[:, :, :qn], in_=st[:, :, :qn],
                                         func=mybir.ActivationFunctionType.Exp,
                                         scale=1.0 / (4.0 * TEMP))
                    for j in range(GK):
                        t = GK * g + j
                        mm(out=y_ps[:, :qn],
                           lhsT=gt_sb[:, t, : I + 1],
                           rhs=e_t[:, j, :qn],
                           start=(t == 0), stop=(t == NKV - 1))
                if ci == 0:
                    emit_gate()
                # epilogue: copy Y out (frees the bank), W-project the
                # UNNORMALIZED Y (so the PE tail never waits on the divide),
                # and fold 1/denominator into the final DVE pass
                ysum = work.tile([I, QCH], BF16, tag="ysum")
                nc.vector.tensor_copy(out=ysum[:, :qn], in_=y_ps[:I, :qn])
                d_sb = work.tile([1, QCH], F32, tag="d")
                nc.vector.tensor_copy(out=d_sb[:, :qn], in_=y_ps[I : I + 1, :qn])
                recip = work.tile([1, QCH], F32, tag="recip")
                nc.vector.reciprocal(out=recip[:, :qn], in_=d_sb[:, :qn])
                bc = work.tile([C, QCH], F32, tag="bc")
                nc.gpsimd.partition_broadcast(bc[:, :qn], recip[:, :qn])

                def _tail(qs=qs, qn=qn, ysum=ysum, bc=bc):
                    o_ps = pmisc.tile([128, QCH], F32, tag="m")
                    mm(out=o_ps[:C, :qn], lhsT=ww_sb, rhs=ysum[:, :qn],
                       start=True, stop=True)
                    t1 = work.tile([C, QCH], F32, tag="t1")
                    nc.vector.tensor_mul(t1[:, :qn], o_ps[:C, :qn], bc[:, :qn])
                    out_sb = work.tile([C, QCH], F32, tag="out")
                    nc.vector.scalar_tensor_tensor(
                        out=out_sb[:, :qn], in0=t1[:, :qn], scalar=gate_sb,
                        in1=xb_sb[:, ds(qs, qn)],
                        op0=mybir.AluOpType.mult, op1=mybir.AluOpType.add)
                    nc.sync.dma_start(out=out_d[:, ds(qs, qn)],
                                      in_=out_sb[:, :qn])

                pending = _tail
            pending()


def build():
    nc = bacc.Bacc("TRN2", target_bir_lowering=False, debug=False)
    names = {
        "xb": ([C, N], F32), "xbh2": ([128, N], BF16),
        "wbf": ([128, 352], BF16), "wf32": ([C, 82], F32),
    }
    dr = {k: nc.dram_tensor(k, shp, dt, kind="ExternalInput").ap()
          for k, (shp, dt) in names.items()}
    out_d = nc.dram_tensor("out", [C, QPC], F32, kind="ExternalOutput").ap()
    with tile.TileContext(nc) as tc:
        _emit(tc, nc, dr, out_d)
    nc.compile()
    return nc


_NC = None


def _get_nc():
    global _NC
    if _NC is None:
        _NC = build()
    return _NC


def make_in_maps(inputs):
    bf = ml_dtypes.bfloat16
    xf = np.ascontiguousarray(np.asarray(inputs["x"], np.float32).reshape(B, C, N))
    thwT = np.asarray(inputs["theta_w"], np.float32).T        # [C, I]
    phwT = np.asarray(inputs["phi_w"], np.float32).T
    gwT = np.asarray(inputs["g_w"], np.float32).T
    wbf = np.zeros((128, 352), np.float32)
    wbf[:, 0:128] = np.tile(thwT, (2, 4)) * 0.5
    wbf[:, 128:256] = np.tile(phwT, (2, 4)) * 0.5
    wbf[:, 256:288] = np.tile(gwT, (2, 1)) * 0.5
    wbf[:I, 288:352] = np.asarray(inputs["W_w"], np.float32).T
    wf32 = np.zeros((C, 82), np.float32)
    wf32[:, 0:NB] = np.asarray(inputs["cg1_w"], np.float32).T
    wf32[:NB, NB] = np.asarray(inputs["cg1_b"], np.float32)
    wf32[:NB, 17:81] = np.asarray(inputs["cg2_w"], np.float32).T
    wf32[:, 81] = -np.asarray(inputs["cg2_b"], np.float32)
    shared = {"wbf": wbf.astype(bf), "wf32": wf32}
    in_maps = []
    for core in range(NCORES):
        b, q0 = core // CPB, (core % CPB) * QPC
        m = dict(shared)
        xr = np.ascontiguousarray(np.roll(xf[b], -q0, axis=1))
        m["xb"] = xr
        m["xbh2"] = np.ascontiguousarray(np.tile(xr, (2, 1))).astype(bf)
        in_maps.append(m)
    return in_maps


def gather(results):
    y = np.empty((B, C, N), np.float32)
    for core in range(NCORES):
        b, q0 = core // CPB, (core % CPB) * QPC
        y[b][:, q0 : q0 + QPC] = results[core]["out"]
    return y.reshape(B, C, H, W)


def run(inputs, trace=False, **kw):
    res = run_bass_kernel_spmd(_get_nc(), make_in_maps(inputs),
                               core_ids=list(range(NCORES)), trace=trace, **kw)
    return gather(res.results), res


def kernel(**inputs):
    out, _ = run(inputs)
    return out
